# revision 1
# baseline (speedup 1.0000x reference)
"""Trainium2 Bass kernel for nn_Ag3SRModel (GNN message passing, 4096 atoms).

reference math:
  d_ij pairwise distances, mask = (d>0) & (d<5)
  rbf_k(d) = exp(-(d - k/3)^2 * 4.5), k=0..15
  features[i,k] = sum_j mask * rbf_k(d_ij)
  e = silu(features @ W1 + b1) @ W2 + b2 ; out = sum(e)

Device strategy (8 NeuronCores, SPMD, row-block over atoms i):
  - per core: 4 i-blocks of 128 atoms (partitions) x all 4096 j (free dim).
  - d^2 via augmented matmul (contraction K=5):
      lhsT = [-2X^T; 1; r][:, i-block]   rhs = [X^T; r; 1][:, j-half]
    into PSUM [128, 2048] halves (4 banks each, 2 bufs = 8 banks).
  - cutoff fold on d^2 per half (DVE, also clamps fp32-negative d^2 to 0):
      d'^2 = max(d^2, 144*(d^2>=25)) ; then d' = sqrt(d'^2) on ACT at 4096.
    masked pairs land at d' in [12, 26] where every rbf_k underflows to
    exactly 0 and exp(3 d') <= exp(78) stays finite.
  - unnormalized rbf chain anchored at k=8, pre-scaled by e^-SHIFT, all
    chain tensors bf16 (DVE tensor_tensor runs 2x in bf16):
      v_8 = exp(-4.5 (d'-8/3)^2 - SHIFT)        (Square+Exp on ACT)
      v_{k+1} = v_k * t,    t    = exp( 3 d')
      v_{k-1} = v_k * tinv, tinv = exp(-3 d')
    v_k = rbf_k / Q_k, Q_k = exp((64-k^2)/2 + SHIFT).
  - j-reductions of v_k split between DVE tensor_reduce and ACT
    Identity+accum_out to balance engine load; k=8 rides the anchor Exp.
  - source order is software-pipelined: tile ib+1's matmul/fold/ACT-core is
    emitted BEFORE tile ib's chains+reductions so ACT never stalls behind
    the DVE chain of the previous tile.
  - device outputs raw feature sums [512, 16]; host scales by Q_k, subtracts
    the diagonal rbf_k(0), runs the tiny MLP, sums energies.
"""

import math
import sys

sys.path.insert(0, "/opt/trn_rl_repo")

import numpy as np

import concourse.bass as bass
import concourse.tile as tile
from concourse import bacc, mybir
from concourse.bass_utils import run_bass_kernel_spmd

N = 4096
NCORES = 8
SLAB = N // NCORES          # 512 atoms i per core
P = 128                     # partitions
NIB = SLAB // P             # 4 i-blocks per core
JF = N                      # all j in one logical tile
HALF = 2048                 # psum half-tile
NRBF = 16
CUTOFF = 5.0
INV2W2 = 4.5                # 1/(2 w^2), w = 1/3
SQ = math.sqrt(INV2W2)
KA = 8                      # anchor k
CA = KA / 3.0
SHIFT = 48 * math.log(2.0)
F32 = mybir.dt.float32
BF16 = mybir.dt.bfloat16

# which k's reduce on DVE tensor_reduce (rest: ACT Identity+accum);
# the last i-block leans slightly DVE; trace-tuned per-tile split
DVE_KS = (0, 4, 10, 14)
DVE_KS_FIRST = (0, 4, 10, 14)
DVE_KS_LAST = (0, 4, 6, 10, 14)

_CACHE = {}


def _build():
    nc = bacc.Bacc("TRN2", target_bir_lowering=False, debug=False,
                   num_devices=NCORES)

    ab_d = nc.dram_tensor("AB", [5, N + SLAB], F32, kind="ExternalInput").ap()
    feats_d = nc.dram_tensor("feats", [SLAB, NRBF], F32, kind="ExternalOutput").ap()

    with tile.TileContext(nc) as tc:
        with (
            tc.tile_pool(name="singles", bufs=1) as singles,
            tc.tile_pool(name="w1", bufs=1) as w1p,
            tc.tile_pool(name="w2", bufs=2) as w2p,
            tc.tile_pool(name="chainp", bufs=8) as chainp,
            tc.tile_pool(name="facc", bufs=4) as faccp,
            tc.tile_pool(name="psum_d2", bufs=2, space="PSUM") as psum_d2,
        ):
            ab_sb = singles.tile([5, N + SLAB], F32)
            nc.sync.dma_start(out=ab_sb, in_=ab_d)
            bias8_sb = singles.tile([P, 1], F32)
            nc.vector.memset(bias8_sb, -SQ * CA)
            biasS_sb = singles.tile([P, 1], F32)
            nc.vector.memset(biasS_sb, -SHIFT)

            dummy = singles.tile([P, 1], BF16)

            def head_fold(ib):
                """matmul + cutoff fold (PE + DVE) for i-block ib."""
                lhsT = ab_sb[:, N + ib * P: N + (ib + 1) * P]
                d2c = w1p.tile([P, JF], F32, tag="d2c")
                for half in range(JF // HALF):
                    d2_ps = psum_d2.tile([P, HALF], F32, tag="d2")
                    for h in range(HALF // 512):
                        c0 = half * HALF + h * 512
                        nc.tensor.matmul(
                            d2_ps[:, h * 512:(h + 1) * 512],
                            lhsT, ab_sb[:, c0:c0 + 512],
                            start=True, stop=True,
                        )
                    m144 = w1p.tile([P, HALF], F32, tag="m144")
                    nc.vector.tensor_scalar(m144, d2_ps, CUTOFF * CUTOFF,
                                            144.0, mybir.AluOpType.is_ge,
                                            mybir.AluOpType.mult)
                    nc.vector.tensor_tensor(
                        d2c[:, half * HALF:(half + 1) * HALF], d2_ps, m144,
                        mybir.AluOpType.max)
                return d2c

            def make_core(ib, d2c):
                """Allocate ACT-core tiles; return emit-closures + tiles."""
                dp = w1p.tile([P, JF], F32, tag="dp")
                s8 = w1p.tile([P, JF], F32, tag="s8")
                t = w2p.tile([P, JF], BF16, tag="t")
                tinv = w2p.tile([P, JF], BF16, tag="tinv")
                v8 = w2p.tile([P, JF], BF16, tag="v8")
                fraw = faccp.tile([P, NRBF], F32, tag="fraw")
                A = mybir.ActivationFunctionType
                ops = [
                    lambda: nc.scalar.activation(dp, d2c, A.Sqrt),
                    lambda: nc.scalar.activation(s8, dp, A.Square,
                                                 bias=bias8_sb, scale=SQ),
                    lambda: nc.scalar.activation(
                        v8, s8, A.Exp, scale=-1.0, bias=biasS_sb,
                        accum_out=fraw[:, KA:KA + 1]),
                    lambda: nc.scalar.activation(t, dp, A.Exp, scale=3.0),
                    lambda: nc.scalar.activation(tinv, dp, A.Exp, scale=-3.0),
                ]
                return ops, (t, tinv, v8, fraw)

            def tail(ib, tiles, next_ops, last=False):
                """chains + reductions for i-block ib, interleaving the next
                i-block's ACT-core ops so ACT serves reductions promptly."""
                t, tinv, v8, fraw = tiles
                dve_ks = (DVE_KS_LAST if last
                          else DVE_KS_FIRST if ib == 0 else DVE_KS)
                inject_at = {1: 0, 4: 1, 7: 2, 9: 3, 11: 4}
                step = 0

                def emit_reduce(k, v):
                    nonlocal step
                    col = fraw[:, k:k + 1]
                    if k in dve_ks:
                        nc.vector.tensor_reduce(col, v,
                                                mybir.AxisListType.X,
                                                mybir.AluOpType.add)
                    else:
                        nc.scalar.activation(
                            dummy.broadcast_to((P, JF)), v,
                            mybir.ActivationFunctionType.Identity,
                            accum_out=col)
                    if next_ops is not None and step in inject_at:
                        next_ops[inject_at[step]]()
                    step += 1

                v = v8
                for k in range(KA + 1, NRBF):       # chain up
                    vn = chainp.tile([P, JF], BF16, tag="vchain")
                    nc.vector.tensor_tensor(vn, v, t, mybir.AluOpType.mult)
                    emit_reduce(k, vn)
                    v = vn
                v = v8
                for k in range(KA - 1, -1, -1):     # chain down
                    vn = chainp.tile([P, JF], BF16, tag="vchain")
                    nc.vector.tensor_tensor(vn, v, tinv,
                                            mybir.AluOpType.mult)
                    emit_reduce(k, vn)
                    v = vn
                nc.sync.dma_start(out=feats_d[ib * P:(ib + 1) * P, :],
                                  in_=fraw)

            # software pipeline: fold(ib+1) precedes chains(ib) on DVE;
            # core(ib+1) ACT ops are interleaved among reductions(ib)
            d2c0 = head_fold(0)
            dp0 = w1p.tile([P, JF], F32, tag="dp")
            s80 = w1p.tile([P, JF], F32, tag="s8")
            t0 = w2p.tile([P, JF], BF16, tag="t")
            tinv0 = w2p.tile([P, JF], BF16, tag="tinv")
            v80 = w2p.tile([P, JF], BF16, tag="v8")
            fraw0 = faccp.tile([P, NRBF], F32, tag="fraw")
            f8b = faccp.tile([P, 1], F32, tag="f8b")
            AF = mybir.ActivationFunctionType
            for h0, h1, acc in ((0, HALF, fraw0[:, KA:KA + 1]),
                                (HALF, JF, f8b)):
                nc.scalar.activation(dp0[:, h0:h1], d2c0[:, h0:h1], AF.Sqrt)
                nc.scalar.activation(s80[:, h0:h1], dp0[:, h0:h1], AF.Square,
                                     bias=bias8_sb, scale=SQ)
                nc.scalar.activation(v80[:, h0:h1], s80[:, h0:h1], AF.Exp,
                                     scale=-1.0, bias=biasS_sb, accum_out=acc)
                nc.scalar.activation(t0[:, h0:h1], dp0[:, h0:h1], AF.Exp,
                                     scale=3.0)
                nc.scalar.activation(tinv0[:, h0:h1], dp0[:, h0:h1], AF.Exp,
                                     scale=-3.0)
            nc.vector.tensor_tensor(fraw0[:, KA:KA + 1], fraw0[:, KA:KA + 1],
                                    f8b, mybir.AluOpType.add)
            prev = (t0, tinv0, v80, fraw0)
            for ib in range(1, NIB):
                opsN, tilesN = make_core(ib, head_fold(ib))
                tail(ib - 1, prev, opsN)
                prev = tilesN
            tail(NIB - 1, prev, None, last=True)

    nc.compile()
    return nc


def kernel(positions, W1, b1, W2, b2):
    positions = np.asarray(positions, dtype=np.float32)
    W1 = np.asarray(W1, dtype=np.float32)
    b1 = np.asarray(b1, dtype=np.float32)
    W2 = np.asarray(W2, dtype=np.float32)
    b2 = np.asarray(b2, dtype=np.float32)

    if "nc" not in _CACHE:
        _CACHE["nc"] = _build()
    nc = _CACHE["nc"]

    r = (positions.astype(np.float64) ** 2).sum(axis=1)
    xt = positions.T.astype(np.float64)                    # [3, N]
    A = np.concatenate([xt, r[None, :], np.ones((1, N))])              # [5, N]
    B = np.concatenate([-2.0 * xt, np.ones((1, N)), r[None, :]])       # [5, N]

    in_maps = [
        {"AB": np.concatenate(
            [A, B[:, c * SLAB:(c + 1) * SLAB]], axis=1).astype(np.float32)}
        for c in range(NCORES)
    ]
    res = run_bass_kernel_spmd(nc, in_maps, list(range(NCORES)))
    feats_raw = np.concatenate([res.results[c]["feats"] for c in range(NCORES)])

    ks = np.arange(NRBF, dtype=np.float64)
    Q = np.exp((KA * KA - ks * ks) / 2.0 + SHIFT)
    ek = np.exp(-0.5 * ks * ks)             # diagonal rbf_k(0)
    f = (feats_raw.astype(np.float64) * Q - ek).astype(np.float32)

    z = (f @ W1 + b1).astype(np.float64)
    h = z * 0.5 * (1.0 + np.tanh(0.5 * z))  # silu, overflow-safe
    e = h @ W2.reshape(-1, 1) + b2.reshape(1, -1)
    return np.float32(e.sum())



# revision 5
# speedup vs baseline: 3.1588x; 3.1588x over previous
"""Trainium2 Bass kernel for nn_Ag3SRModel (GNN message passing, 4096 atoms).

reference math:
  d_ij pairwise distances, mask = (d>0) & (d<5)
  rbf_k(d) = exp(-(d - k/3)^2 * 4.5), k=0..15
  features[i,k] = sum_j mask * rbf_k(d_ij)
  e = silu(features @ W1 + b1) @ W2 + b2 ; out = sum(e)

Strategy (8 NeuronCores, SPMD):
  The seed-0 positions are spatially clustered, so a kd-tree (median split,
  longest axis) gives 32 tiles of exactly 128 atoms with tight bboxes. For
  each tile the host computes the exact candidate set {j : min_i d_ij < 5}
  (~450-1130 atoms, vs 4096 dense) and packs it into [128 i x 512 j] chunks
  (last chunk padded with a far point P0, >=5A from every real atom so the
  cutoff fold zeroes it). 61 real + 3 dummy chunks = 8 chunks x 8 cores,
  each chunk an independent unit: per-chunk partial features are summed on
  the host.

  Per chunk on device:
    d^2 via augmented matmul (contraction K=5) into a PSUM bank
    cutoff fold on DVE: d'^2 = max(d^2, 144*(d^2>=25))  (masked d' in
      [12,~24] where every rbf underflows to exactly 0)
    d = sqrt(d'^2) (ACT), t = exp(3d) bf16 (ACT)
    anchors k in {0,2,4,6,8,10,12}: one ACT Derivative_Erf pass each:
      a_k = (2/sqrt(pi)) * exp(-(sqrt(4.5) d - sqrt(4.5) c_k)^2)
      with accum_out -> feature column k reduced for free
    k in {1,3,5,7,9,11} + chain 13,14,15: one DVE affine_mul_reduce each:
      out = (prev * alpha_k) * t,  accum_out = sum_j out
      alpha_k = exp(-(2k-1)/2) keeps every tile at true-rbf magnitude
  ACT table thrash is avoided by phase-batching chunks in groups of 4:
  [sqrt x4][exp x4][derf/amr x4] costs 3 table loads per group.

  Host: sum chunk partials, scale by sqrt(pi)/2, subtract diagonal rbf_k(0),
  tiny MLP in f64, total energy.
"""

import math
import sys

sys.path.insert(0, "/opt/trn_rl_repo")

import numpy as np

import concourse.bass as bass
import concourse.tile as tile
from concourse import bacc, mybir
from concourse.bass_utils import run_bass_kernel_spmd

N = 4096
NCORES = 8
P = 128                     # partitions / atoms per kd tile
CW = 512                    # chunk width (j columns)
NRBF = 16
CUTOFF = 5.0
INV2W2 = 4.5                # 1/(2 w^2), w = 1/3
SQ = math.sqrt(INV2W2)
GAUSS_NORM = math.sqrt(math.pi) / 2.0   # undo derivative_erf's 2/sqrt(pi)
ANCHORS = (0, 2, 4, 6, 8, 10, 12)
PAD_POINT = np.array([7.5, 7.5, 21.0])  # >=5A from box, <=23.6A away
GROUP = 4                   # chunks per ACT-table phase group
F32 = mybir.dt.float32
BF16 = mybir.dt.bfloat16

_CACHE = {}


def _build(K):
    """Device program: K independent [128 x 512] chunks per core."""
    nc = bacc.Bacc("TRN2", target_bir_lowering=False, debug=False,
                   num_devices=NCORES)

    # per chunk: 128 lhsT cols + 512 rhs cols
    ab_d = nc.dram_tensor("AB", [5, K * (P + CW)], F32,
                          kind="ExternalInput").ap()
    feats_d = nc.dram_tensor("feats", [K * P, NRBF], F32,
                             kind="ExternalOutput").ap()

    A = mybir.ActivationFunctionType
    ALU = mybir.AluOpType

    with tile.TileContext(nc) as tc:
        with (
            tc.tile_pool(name="singles", bufs=1) as singles,
            tc.tile_pool(name="dtile", bufs=GROUP + 2) as dpool,
            tc.tile_pool(name="ttile", bufs=GROUP + 2) as tpool,
            tc.tile_pool(name="d2c", bufs=GROUP + 1) as d2cpool,
            tc.tile_pool(name="m144", bufs=2) as mpool,
            tc.tile_pool(name="anch", bufs=4) as apool,
            tc.tile_pool(name="scr", bufs=2) as spool,
            tc.tile_pool(name="fraw", bufs=GROUP + 2) as fpool,
            tc.tile_pool(name="psum_d2", bufs=8, space="PSUM") as psum_d2,
        ):
            ab_sb = singles.tile([5, K * (P + CW)], F32)
            nc.sync.dma_start(out=ab_sb, in_=ab_d)
            biases = {}
            for k in ANCHORS:
                if k == 0:
                    continue
                b = singles.tile([P, 1], F32, tag=f"bias{k}")
                nc.vector.memset(b, -SQ * (k / 3.0))
                biases[k] = b

            def off(c):
                return c * (P + CW)

            # ---- stage 1: all matmuls (PE), then folds (DVE) per group ----
            def mm(c):
                ps = psum_d2.tile([P, CW], F32, tag="d2")
                nc.tensor.matmul(ps, ab_sb[:, off(c):off(c) + P],
                                 ab_sb[:, off(c) + P:off(c + 1)],
                                 start=True, stop=True)
                return ps

            def fold(c, ps):
                m = mpool.tile([P, CW], F32, tag="m144")
                nc.vector.tensor_scalar(m, ps, CUTOFF * CUTOFF, 144.0,
                                        ALU.is_ge, ALU.mult)
                d2c = d2cpool.tile([P, CW], F32, tag="d2c")
                nc.vector.tensor_tensor(d2c, ps, m, ALU.max)
                return d2c

            def act_core(c, d2c):
                d = dpool.tile([P, CW], F32, tag="d")
                nc.scalar.activation(d, d2c, A.Sqrt)
                return d

            def act_exp(c, d):
                t = tpool.tile([P, CW], BF16, tag="t")
                nc.scalar.activation(t, d, A.Exp, scale=3.0)
                return t

            def chunk_tail(c, d, t):
                """7 derf anchors (ACT) + 9 amr chain steps (DVE)."""
                fraw = fpool.tile([P, NRBF], F32, tag="fraw")
                anch = {}
                for k in ANCHORS:
                    ak = apool.tile([P, CW], BF16, tag="anch")
                    kw = {} if k == 0 else {"bias": biases[k]}
                    nc.scalar.activation(ak, d, A.Derivative_Erf, scale=SQ,
                                         accum_out=fraw[:, k:k + 1], **kw)
                    anch[k] = ak
                    if k + 1 <= 11:
                        out = spool.tile([P, CW], BF16, tag="scr")
                        nc.vector.affine_mul_reduce(
                            out, fraw[:, k + 1:k + 2], ak, t,
                            scale=math.exp(-(2 * (k + 1) - 1) / 2.0),
                            bias=0.0)
                # chain 13, 14, 15 off anchor 12
                prev = anch[12]
                for k in (13, 14, 15):
                    out = spool.tile([P, CW], BF16, tag="scr")
                    nc.vector.affine_mul_reduce(
                        out, fraw[:, k:k + 1], prev, t,
                        scale=math.exp(-(2 * k - 1) / 2.0), bias=0.0)
                    prev = out
                nc.sync.dma_start(out=feats_d[c * P:(c + 1) * P, :], in_=fraw)

            # software pipeline in groups of GROUP chunks:
            # PE runs ahead; ACT does [sqrt xG][exp xG][derf...] per group
            # so tables load 3x per group; DVE folds next group during the
            # derf/amr phase of the current one.
            psums = {}
            d2cs = {}
            ds = {}
            ts = {}
            for c in range(min(2 * GROUP, K)):
                psums[c] = mm(c)
            for c in range(min(GROUP, K)):
                d2cs[c] = fold(c, psums.pop(c))
            g = 0
            while g * GROUP < K:
                lo, hi = g * GROUP, min((g + 1) * GROUP, K)
                nlo, nhi = hi, min(hi + GROUP, K)
                for c in range(lo, hi):
                    ds[c] = act_core(c, d2cs.pop(c))
                for c in range(lo, hi):
                    ts[c] = act_exp(c, ds[c])
                # fold next group now (DVE) so ACT never waits on it
                for c in range(nlo, nhi):
                    d2cs[c] = fold(c, psums.pop(c))
                for c in range(nlo + GROUP, nhi + GROUP):
                    if c < K:
                        psums[c] = mm(c)
                for c in range(lo, hi):
                    chunk_tail(c, ds.pop(c), ts.pop(c))
                g += 1

    nc.compile()
    return nc


def _kdtiles(pos, idx):
    if len(idx) <= P:
        return [idx]
    p = pos[idx]
    ax = int(np.argmax(p.max(0) - p.min(0)))
    o = np.argsort(p[:, ax], kind="stable")
    h = len(idx) // 2
    return _kdtiles(pos, idx[o[:h]]) + _kdtiles(pos, idx[o[h:]])


def _prep(positions):
    """kd tiling + exact candidate sets + chunk packing."""
    pos = positions.astype(np.float64)
    tiles = _kdtiles(pos, np.arange(len(pos)))
    chunks = []          # (tile_atom_idx [128], j_atom_idx [<=512])
    for tidx in tiles:
        p = pos[tidx]
        bd2 = (np.maximum(0.0, np.maximum(p.min(0)[None] - pos,
                                          pos - p.max(0)[None])) ** 2).sum(1)
        cand = np.where(bd2 < CUTOFF * CUTOFF)[0]
        d2 = ((pos[cand][:, None, :] - p[None, :, :]) ** 2).sum(-1)
        cand = cand[(d2 < CUTOFF * CUTOFF).any(1)]
        for s in range(0, len(cand), CW):
            chunks.append((tidx, cand[s:s + CW]))
    K = (len(chunks) + NCORES - 1) // NCORES
    # greedy: distribute chunks round-robin (they are near-uniform cost)
    percore = [[] for _ in range(NCORES)]
    for i, ch in enumerate(chunks):
        percore[i % NCORES].append(ch)
    return K, percore


def _pack(pos, percore, K):
    """Build per-core AB input arrays + chunk->atom scatter metadata."""
    pos64 = pos.astype(np.float64)
    in_maps, meta = [], []
    for c in range(NCORES):
        ab = np.zeros((5, K * (P + CW)), dtype=np.float64)
        mm = []
        for s in range(K):
            o = s * (P + CW)
            if s < len(percore[c]):
                tidx, jidx = percore[c][s]
                ti = pos64[tidx]                       # [128, 3]
                tj = pos64[jidx]                       # [<=512, 3]
                nj = len(jidx)
                ab[0:3, o:o + P] = -2.0 * ti.T
                ab[3, o:o + P] = 1.0
                ab[4, o:o + P] = (ti ** 2).sum(1)
                ab[0:3, o + P:o + P + nj] = tj.T
                ab[3, o + P:o + P + nj] = (tj ** 2).sum(1)
                ab[4, o + P:o + P + nj] = 1.0
                if nj < CW:
                    ab[0:3, o + P + nj:o + P + CW] = PAD_POINT[:, None]
                    ab[3, o + P + nj:o + P + CW] = (PAD_POINT ** 2).sum()
                    ab[4, o + P + nj:o + P + CW] = 1.0
                mm.append(tidx)
            else:                                      # dummy chunk
                ab[0:3, o:o + P] = PAD_POINT[:, None]
                ab[3, o:o + P] = 1.0
                ab[4, o:o + P] = (PAD_POINT ** 2).sum()
                ab[0:3, o + P:o + P + CW] = PAD_POINT[:, None]
                ab[3, o + P:o + P + CW] = (PAD_POINT ** 2).sum()
                ab[4, o + P:o + P + CW] = 1.0
                mm.append(None)
        in_maps.append({"AB": ab.astype(np.float32)})
        meta.append(mm)
    return in_maps, meta


def kernel(positions, W1, b1, W2, b2):
    positions = np.asarray(positions, dtype=np.float32)
    W1 = np.asarray(W1, dtype=np.float32)
    b1 = np.asarray(b1, dtype=np.float32)
    W2 = np.asarray(W2, dtype=np.float32)
    b2 = np.asarray(b2, dtype=np.float32)

    K, percore = _prep(positions)
    if ("nc", K) not in _CACHE:
        _CACHE[("nc", K)] = _build(K)
    nc = _CACHE[("nc", K)]
    _CACHE["last"] = (nc, K, percore)

    in_maps, meta = _pack(positions, percore, K)
    res = run_bass_kernel_spmd(nc, in_maps, list(range(NCORES)))

    feats = np.zeros((N, NRBF), dtype=np.float64)
    for c in range(NCORES):
        fr = res.results[c]["feats"].astype(np.float64)   # [K*128, 16]
        for s, tidx in enumerate(meta[c]):
            if tidx is not None:
                feats[tidx] += fr[s * P:(s + 1) * P]
    feats *= GAUSS_NORM

    ks = np.arange(NRBF, dtype=np.float64)
    ek = np.exp(-0.5 * ks * ks)             # diagonal rbf_k(0)
    f = (feats - ek).astype(np.float32)

    z = (f @ W1 + b1).astype(np.float64)
    h = z * 0.5 * (1.0 + np.tanh(0.5 * z))  # silu, overflow-safe
    e = h @ W2.reshape(-1, 1) + b2.reshape(1, -1)
    return np.float32(e.sum())
